# revision 19
# baseline (speedup 1.0000x reference)
"""Trainium2 Bass kernel: cross multi-head attention (bf16 datapath).

Problem shape: B=8, S=1024, D=1024, H=16 heads (head dim 64), fp32 I/O.

    x  = encoding_output @ Wqk + bqk          -> split into q, k per head
    v  = input_query @ Wv + bv
    out = softmax(q k^T / sqrt(64)) v @ Wo + bo

Sharding: data-parallel over batch. Core i computes batch element i
end-to-end (no collectives); weights are replicated to all 8 cores.

Differences vs the f32r baseline (410us):
  * Whole on-chip datapath in bf16 (inputs/weights converted once on
    load); PSUM accumulation stays fp32, final output fp32. Matmul
    cycles/row match f32r at N>=256, but bf16 also runs 1 cycle/row at
    small N, halves SBUF footprint and DMA bytes, and makes PE
    transposes 1.0 cycles/row.
  * attn@V flipped: stationary = eT 128x128 block, moving = [V_h|ones]
    (65 cols). 64x65-col matmuls per head instead of 16x512 -> ~2.4x
    fewer PE cycles for this stage. Softmax denominators land in PSUM
    col 64 per sq row, so normalization is a per-partition
    reciprocal+scale (no DRAM round-trip broadcast).
  * Single flat pool scope; head pipeline is software-pipelined two
    heads ahead; phase A (transpose + V projection) interleaved with
    heads 0-1 so ScalarE exp work starts early.

This session's optimizations (407us -> ~250us measured marginal HW time):
  * Scores matmuls run with K=128 instead of K=64: stationary is the
    full qk_sb (q rows 0:63, k rows 64:127) and the moving q2 operand
    zeroes partitions 0:63, so the q-half of the stationary multiplies
    zeros. HW microbench: the PE streams ~1.5x faster with a full
    128-deep contraction (0.27ns/row vs 0.65ns/row at K=64) - the zero
    rows are numerically exact, not an approximation.
  * exp runs as one [128,1024]-wide Activation per skt block (128
    instead of 256 dispatches); score PSUM tiles widened to 2 banks,
    paid for by sharing the transpose PSUM slot with the AV pool.
  * Wo prefetch moved off the gpsimd ring (it was queued ahead of the
    per-head q2 copies that gate scores) to the sync ring, which is
    idle in steady state, and issued earlier (h=8/h=10).
  * xq input stream moved to the gpsimd ring so both 8MB input streams
    transfer in parallel during fill.
Bottleneck after these changes: ScalarE (Activation) exp throughput,
~130us of the ~250us total (16.8M exps at ~1ns/elem f32-PSUM-in);
PE ~150us, fill ~30us, tail ~30us, rest is imperfect overlap.
"""

import sys

for _p in ("/opt/trn_rl_repo",):
    if _p not in sys.path:
        sys.path.insert(0, _p)

import numpy as np

import concourse.bass as bass
from concourse import bacc
import concourse.mybir as mybir
import concourse.tile as tile
from concourse.masks import make_identity

B, S, D = 8, 1024, 1024
H = 16
HD = D // H          # 64, head dim
P = 128              # SBUF partitions
NCH = D // P         # 8 chunks of the d/j dims
SCH = S // P         # 8 chunks of s
NH = 512             # matmul moving free-dim (PSUM bank)
VW = HD + 1          # 65: [v_h | ones] window per head
VROW = H * VW        # 1040
SCALE = float(1.0 / np.sqrt(np.float32(HD)))

F32 = mybir.dt.float32
BF16 = mybir.dt.bfloat16
EXP = mybir.ActivationFunctionType.Exp


def _ap(base, off, dims):
    """Custom free-dim AP over `base` (a [128, F] AP): partition dim kept."""
    return bass.AP(
        tensor=base.tensor,
        offset=base.offset + off,
        ap=[list(base.ap[0])] + [list(d) for d in dims],
    )


def build_nc(reps=1):
    nc = bacc.Bacc()

    xq_t = nc.dram_tensor("input_query", [S, D], F32, kind="ExternalInput")
    xe_t = nc.dram_tensor("encoding_output", [S, D], F32, kind="ExternalInput")
    wqk_t = nc.dram_tensor("Wqk", [D, 2 * D], F32, kind="ExternalInput")
    bqk_t = nc.dram_tensor("bqk", [2 * D], F32, kind="ExternalInput")
    wv_t = nc.dram_tensor("Wv", [D, D], F32, kind="ExternalInput")
    bv_t = nc.dram_tensor("bv", [D], F32, kind="ExternalInput")
    wo_t = nc.dram_tensor("Wo", [D, D], F32, kind="ExternalInput")
    bo_t = nc.dram_tensor("bo", [D], F32, kind="ExternalInput")
    out_t = nc.dram_tensor("out", [S, D], F32, kind="ExternalOutput")

    with tile.TileContext(nc) as tc:
        for _rep in range(reps):
            _build_kernel(tc, xq_t, xe_t, wqk_t, bqk_t, wv_t, bv_t, wo_t, bo_t, out_t)
    nc.compile()
    return nc

def _build_kernel(tc, xq_t, xe_t, wqk_t, bqk_t, wv_t, bv_t, wo_t, bo_t, out_t):
    nc = tc.nc

    with (
        tc.tile_pool(name="singles", bufs=1) as singles,
        tc.tile_pool(name="bigs", bufs=1) as bigs,
        tc.tile_pool(name="ld", bufs=2) as ldp,
        tc.tile_pool(name="cv", bufs=2) as cvp,
        tc.tile_pool(name="xqTp", bufs=1) as pxq,
        tc.tile_pool(name="wstg", bufs=1) as wstgp,
        tc.tile_pool(name="wbig", bufs=1) as wbigp,
        tc.tile_pool(name="wqf", bufs=2) as wqfp,
        tc.tile_pool(name="wqb", bufs=2) as wqbp,
        tc.tile_pool(name="qkp", bufs=2) as qkp,
        tc.tile_pool(name="etp", bufs=3) as etp,
        tc.tile_pool(name="rsp", bufs=4) as rsp,
        tc.tile_pool(name="finp", bufs=3) as finp,
        tc.tile_pool(name="pacc", bufs=2, space="PSUM") as paccp,
        tc.tile_pool(name="pqkp", bufs=1, space="PSUM") as pqkp,
        tc.tile_pool(name="po", bufs=1, space="PSUM") as pop,
    ):
        # transposes share the po pool (phase A + the h%2 OT slots);
        # pacc holds full-width [128,1024] score tiles so exp runs as one
        # wide Activation instruction per skt block (fewer ACT dispatches)
        pstp = pop
        ident_bf = singles.tile([P, P], BF16)
        make_identity(nc, ident_bf)
        # bqk_all[p, h] = bqk[h*128 + p] - per-partition bias for qkT layout
        bqk_all = singles.tile([P, H], F32)
        nc.gpsimd.dma_start(
            out=bqk_all, in_=bass.AP(tensor=bqk_t, offset=0, ap=[[1, P], [P, H]])
        )
        bv_bc = singles.tile([P, D], F32)
        bo_bc = singles.tile([P, D], F32)

        # xeT[p, dc, s] = xe[s, dc*128 + p], bf16
        xeT = bigs.tile([P, NCH, S], BF16)
        # V_sb[p, sc, h*65 .. h*65+64] = [v_h | ones] per head
        V_sb = bigs.tile([P, SCH, VROW], BF16)
        # O_nat[p, sc, j]: normalized attention output, natural layout
        O_nat = bigs.tile([P, SCH, D], BF16)
        # OT[p, jc, s] = O[s, jc*128 + p]
        OT = bigs.tile([P, NCH, S], BF16)

        # ones columns of V_sb (col 64 of each head window)
        for st in range(SCH):
            nc.gpsimd.memset(_ap(V_sb[:, st, :], HD, [[VW, H]]), 1.0)

        xqT = pxq.tile([P, NCH, S], BF16)

        wqk_r = wqk_t[:].rearrange("(dc p) j -> p dc j", p=P)
        wv_r = wv_t[:].rearrange("(dc p) j -> p dc j", p=P)
        wo_r = wo_t[:].rearrange("(jc p) n -> p jc n", p=P)

        qk_state = {}
        eT_state = {}

        # ---------- emit helpers ----------

        def emit_in_chunk(src_t, dstT, c, via_sync=True):
            """DMA chunk c (xe on the sync ring, xq on the gpsimd ring so
            the two 8MB input streams transfer in parallel; per-stream
            ld-pool tags so pool pacing doesn't couple the rings),
            convert to bf16 (Act), transpose 8x (PE), one fused
            PSUM->SBUF copy (DVE, 2x bf16 mode)."""
            nm = dstT.tensor.name
            xnat = ldp.tile([P, D], F32, tag="xnat", name=f"xn{nm}{c}")
            eng = nc.sync if via_sync else nc.gpsimd
            eng.dma_start(out=xnat, in_=src_t[c * P : (c + 1) * P, :])
            xbf = cvp.tile([P, D], BF16, tag="xbf", name=f"xb{nm}{c}")
            nc.scalar.copy(xbf, xnat)
            pt = pstp.tile([P, S], BF16, tag="pav", name=f"pt{nm}{c}")
            for dc in range(NCH):
                nc.tensor.transpose(
                    pt[:, dc * P : (dc + 1) * P],
                    xbf[:, dc * P : (dc + 1) * P],
                    ident_bf,
                )
            nc.vector.tensor_copy(
                dstT[:, :, c * P : (c + 1) * P],
                pt.rearrange("p (a b) -> p a b", a=NCH),
            )

        def emit_w_half(wb, w_r, nh, nm, via_sync=False):
            wf = wstgp.tile([P, NCH, NH], F32, tag="wstg", name=f"{nm}{nh}")
            eng = nc.sync if via_sync else nc.gpsimd
            eng.dma_start(out=wf, in_=w_r[:, :, nh * NH : (nh + 1) * NH])
            nc.vector.tensor_copy(wb[:, :, nh * NH : (nh + 1) * NH], wf)

        def emit_vproj(wvb, nh, sts):
            """Two s-tiles of the V projection into one [P,S] PSUM tile."""
            pv = pqkp.tile([P, S], F32, tag="pqk", name=f"pv{nh}_{sts[0]}")
            for i, st in enumerate(sts):
                for dc in range(NCH):
                    nc.tensor.matmul(
                        pv[:, i * NH : (i + 1) * NH],
                        xqT[:, dc, st * P : (st + 1) * P],
                        wvb[:, dc, nh * NH : (nh + 1) * NH],
                        start=(dc == 0),
                        stop=(dc == NCH - 1),
                    )
            for i, st in enumerate(sts):
                dst = _ap(
                    V_sb[:, st, :],
                    nh * NCH * VW,
                    [[VW, NCH], [1, HD]],
                )
                nc.vector.tensor_add(
                    dst,
                    pv[:, i * NH : (i + 1) * NH].rearrange("p (a b) -> p a b", a=NCH),
                    bv_bc[:, nh * NH : (nh + 1) * NH].rearrange(
                        "p (a b) -> p a b", a=NCH
                    ),
                )

        def emit_wqk_load(h, via_sync=False):
            wqf = wqfp.tile([P, NCH, P], F32, tag="wqf", name=f"wqf{h}")
            eng = nc.sync if via_sync else nc.gpsimd
            eng.dma_start(out=wqf, in_=wqk_r[:, :, h * P : (h + 1) * P])
            return wqf

        def emit_qkt(h, wqf):
            wqb = wqbp.tile([P, NCH, P], BF16, tag="wqb", name=f"wqb{h}")
            nc.vector.tensor_copy(wqb, wqf)
            pqk = pqkp.tile([P, S], F32, tag="pqk", name=f"pqk{h}")
            for sh in range(2):
                for dc in range(NCH):
                    nc.tensor.matmul(
                        pqk[:, sh * NH : (sh + 1) * NH],
                        wqb[:, dc, :],
                        xeT[:, dc, sh * NH : (sh + 1) * NH],
                        start=(dc == 0),
                        stop=(dc == NCH - 1),
                    )
            qk_sb = qkp.tile([P, S], BF16, tag="qk", name=f"qk{h}")
            nc.vector.tensor_scalar_add(qk_sb, pqk, bqk_all[:, h : h + 1])
            # moving operand of scores: q on partitions 64-127, ZEROS on
            # 0-63 so the scores matmul can run with K=128 (the PE streams
            # ~1.5x faster with a full contraction dim than at K=64; the
            # zero rows multiply the q-halves of the stationary to nothing).
            q2 = qkp.tile([P, S], BF16, tag="q2", name=f"q2{h}")
            nc.gpsimd.memset(q2[0:64, :], 0.0)
            nc.gpsimd.dma_start(out=q2[64:128, :], in_=qk_sb[0:64, :])
            qk_state[h] = (qk_sb, q2)

        def emit_scores_exp(h):
            qk_sb, q2 = qk_state[h]
            ets = etp.tile([P, SCH, S], BF16, tag="eT", name=f"eT{h}")
            for skt in range(SCH):
                ps = paccp.tile([P, S], F32, tag="pacc", name=f"ps{h}_{skt}")
                for sh in range(2):
                    nc.tensor.matmul(
                        ps[:, sh * NH : (sh + 1) * NH],
                        qk_sb[:, skt * P : (skt + 1) * P],
                        q2[:, sh * NH : (sh + 1) * NH],
                        start=True,
                        stop=True,
                    )
                nc.scalar.activation(ets[:, skt, :], ps, EXP, scale=SCALE)
            eT_state[h] = ets

        def emit_av_norm(h):
            ets = eT_state.pop(h)
            qk_state.pop(h)
            pot = pop.tile([P, S], F32, tag="pav", name=f"pav{h}")
            for q in range(SCH):
                off = (q // 4) * NH + (q % 4) * VW
                out_ap = _ap(pot[:], off, [[1, VW]])
                for skc in range(SCH):
                    nc.tensor.matmul(
                        out_ap,
                        ets[:, skc, q * P : (q + 1) * P],
                        V_sb[:, skc, h * VW : (h + 1) * VW],
                        start=(skc == 0),
                        stop=(skc == SCH - 1),
                    )
            # norms after all matmuls: the strided PSUM APs above defeat
            # subtile dependency tracking, so interleaving would serialize
            # each q-block's matmuls behind the previous block's reads.
            # One strided reciprocal (all 8 sums) + one strided multiply.
            rs = rsp.tile([P, SCH], F32, tag="rs", name=f"rs{h}")
            nc.vector.reciprocal(
                rs, _ap(pot[:], HD, [[NH, 2], [VW, 4]])
            )
            nc.vector.tensor_tensor(
                O_nat[:, :, h * HD : (h + 1) * HD],
                _ap(pot[:], 0, [[NH, 2], [VW, 4], [1, HD]]),
                _ap(rs[:], 0, [[1, SCH], [0, HD]]),
                mybir.AluOpType.mult,
            )

        def emit_o_transpose(hp):
            pt = pstp.tile([P, S], BF16, tag="pav", name=f"ot{hp}")
            for c in range(SCH):
                nc.tensor.transpose(
                    pt[:, c * P : (c + 1) * P],
                    O_nat[:, c, hp * P : (hp + 1) * P],
                    ident_bf,
                )
            nc.vector.tensor_copy(OT[:, hp, :], pt)

        def emit_cproj(wob, nh, st):
            pf = paccp.tile([P, NH], F32, tag="pacc", name=f"pf{nh}_{st}")
            for jc in range(NCH):
                nc.tensor.matmul(
                    pf,
                    OT[:, jc, st * P : (st + 1) * P],
                    wob[:, jc, nh * NH : (nh + 1) * NH],
                    start=(jc == 0),
                    stop=(jc == NCH - 1),
                )
            fin = finp.tile([P, NH], F32, tag="fin", name=f"fin{nh}_{st}")
            nc.vector.tensor_add(fin, pf, bo_bc[:, nh * NH : (nh + 1) * NH])
            nc.sync.dma_start(
                out=out_t[st * P : (st + 1) * P, nh * NH : (nh + 1) * NH],
                in_=fin,
            )

        # ---------- emission schedule ----------
        # DMA ordering: the shared DMA engines serialize transfers, so the
        # sync ring carries everything whose order matters (inputs first,
        # then weights woven between input chunks via ld-pool gating);
        # head-0/1 Wqk slices go out up front on the gpsimd ring.

        wqf0 = emit_wqk_load(0)
        wqf1 = emit_wqk_load(1)

        wvb = wbigp.tile([P, NCH, D], BF16, tag="wbig", name="wvb")

        # head 0 qkT interleaved with the xe chunk stream: the sh=0 half
        # of the projection only reads xeT columns from chunks 0-3
        wqb0 = wqbp.tile([P, NCH, P], BF16, tag="wqb", name="wqb0")
        pqk0 = pqkp.tile([P, S], F32, tag="pqk", name="pqk0")
        for c in range(4):
            emit_in_chunk(xe_t, xeT, c)
        nc.vector.tensor_copy(wqb0, wqf0)
        for dc in range(NCH):
            nc.tensor.matmul(
                pqk0[:, 0:NH],
                wqb0[:, dc, :],
                xeT[:, dc, 0:NH],
                start=(dc == 0),
                stop=(dc == NCH - 1),
            )
        for c in range(4, SCH):
            emit_in_chunk(xe_t, xeT, c)
        for dc in range(NCH):
            nc.tensor.matmul(
                pqk0[:, NH:S],
                wqb0[:, dc, :],
                xeT[:, dc, NH:S],
                start=(dc == 0),
                stop=(dc == NCH - 1),
            )
        qk_sb0 = qkp.tile([P, S], BF16, tag="qk", name="qk0")
        nc.vector.tensor_scalar_add(qk_sb0, pqk0, bqk_all[:, 0:1])
        q2_0 = qkp.tile([P, S], BF16, tag="q2", name="q2_0")
        nc.gpsimd.memset(q2_0[0:64, :], 0.0)
        nc.gpsimd.dma_start(out=q2_0[64:128, :], in_=qk_sb0[0:64, :])
        qk_state[0] = (qk_sb0, q2_0)

        emit_in_chunk(xq_t, xqT, 0, via_sync=False)
        wqf2 = emit_wqk_load(2, via_sync=True)
        for c in range(1, 4):
            emit_in_chunk(xq_t, xqT, c, via_sync=False)
        emit_scores_exp(0)
        emit_w_half(wvb, wv_r, 0, "wvf", via_sync=True)
        for c in range(4, SCH):
            emit_in_chunk(xq_t, xqT, c, via_sync=False)
        nc.sync.dma_start(
            out=bv_bc, in_=bass.AP(tensor=bv_t, offset=0, ap=[[0, P], [1, D]])
        )
        emit_qkt(1, wqf1)
        emit_w_half(wvb, wv_r, 1, "wvf", via_sync=True)
        emit_scores_exp(1)
        nc.sync.dma_start(
            out=bo_bc, in_=bass.AP(tensor=bo_t, offset=0, ap=[[0, P], [1, D]])
        )
        emit_qkt(2, wqf2)
        emit_scores_exp(2)
        wqf3 = emit_wqk_load(3, via_sync=True)

        for nh in range(2):
            for sp in range(0, SCH, 2):
                emit_vproj(wvb, nh, (sp, sp + 1))

        # steady-state head pipeline: AV(h) | qkT(h+3) | scores(h+3)
        wqf_next = wqf3
        wob = None
        for h in range(H):
            hn = h + 3
            if hn < H:
                wqf = wqf_next
                wqf_next = emit_wqk_load(hn + 1) if hn + 1 < H else None
                emit_qkt(hn, wqf)
            emit_av_norm(h)
            if h % 2 == 1:
                emit_o_transpose(h // 2)
            if hn < H:
                emit_scores_exp(hn)
            # prefetch Wo mid-pipeline on the sync ring (idle in steady
            # state; the gpsimd ring carries the per-head q2 copies that
            # gate scores, so Wo there would stall heads behind 8MB).
            if h == H - 8:
                wob = wbigp.tile([P, NCH, D], BF16, tag="wbig", name="wob")
                emit_w_half(wob, wo_r, 0, "wof", via_sync=True)
            elif h == H - 6:
                emit_w_half(wob, wo_r, 1, "wof", via_sync=True)

        # output projection
        for nh in range(2):
            for st in range(SCH):
                emit_cproj(wob, nh, st)


_NC_CACHE = None


def _get_nc():
    global _NC_CACHE
    if _NC_CACHE is None:
        _NC_CACHE = build_nc()
    return _NC_CACHE


def make_in_maps(inputs):
    ins = {k: np.ascontiguousarray(np.asarray(v), dtype=np.float32) for k, v in inputs.items()}
    in_maps = []
    for i in range(B):
        in_maps.append(
            dict(
                input_query=ins["input_query"][i],
                encoding_output=ins["encoding_output"][i],
                Wqk=ins["Wqk"],
                bqk=ins["bqk"],
                Wv=ins["Wv"],
                bv=ins["bv"],
                Wo=ins["Wo"],
                bo=ins["bo"],
            )
        )
    return in_maps


def kernel(**inputs):
    from concourse.bass_utils import run_bass_kernel_spmd

    nc = _get_nc()
    res = run_bass_kernel_spmd(nc, make_in_maps(inputs), list(range(B)))
    return np.stack([res.results[i]["out"] for i in range(B)], axis=0).astype(np.float32)


if __name__ == "__main__":
    nc = build_nc()
    print("built OK")

